# revision 1
# baseline (speedup 1.0000x reference)
"""Trainium2 Bass kernel for nn_KVCacheHybrid (quantized KV-cache scatter-update).

Reference semantics (per cache, k and v independently):
  1. 4-bit affine quantize along L (scales/zeros reduce over B,H,D per l)
  2. dequantize, scatter new rows at input_pos, re-quantize, dequantize.

Key observations that shape this kernel:
  * After the first quantize/dequant round-trip, codes 0 and 15 are attained in
    every l-slice, so the second-pass min/max for non-updated l are exactly the
    dequant grid endpoints: mn2 = z1 - 8*s1, mx2 = z1 + 7*s1.  No second data
    reduction is needed.
  * For non-updated l the second-pass codes equal the first-pass codes, so
    out = q1 * s2 + mn2.  Per element the device only computes
    q1 = round((x - mn1) * (1/s1)) and the affine above.
  * Rows at input_pos depend only on k_val/v_val (0.5 MB) — computed exactly on
    the host and spliced into the gathered output.

Sharding: L axis across 8 cores (512 l's each).  The per-l reduction is then
fully core-local — no collectives.

Device layout: partition dim = l (128 per chunk), free dim = (16 heads x 128 d)
=> [128, 2048] fp32 tiles, 1 MiB DMAs.
"""

import numpy as np
from contextlib import ExitStack

import concourse.bass as bass
import concourse.bacc as bacc
import concourse.tile as tile
from concourse import mybir
from concourse.bass_utils import run_bass_kernel_spmd

F32 = mybir.dt.float32
ALU = mybir.AluOpType
AXIS = mybir.AxisListType
ACTF = mybir.ActivationFunctionType

B, H, L, D = 2, 32, 4096, 128
N_CORES = 8
LC = L // N_CORES          # 512 l's per core
LCHUNK = 128               # l's per partition-tile
HG = 16                    # heads per tile (free dim = HG*D = 2048)
MAGIC = float(np.float32(2 ** 23))   # round-to-nearest-even constant
C15 = float(np.float32(1.0 / 15.0))

_BUILD_CACHE = {}


def _build(lc=LC):
    """Builds the per-core SPMD program; identical on all cores."""
    nc = bacc.Bacc("TRN2", target_bir_lowering=False, debug=False,
                   num_devices=N_CORES)
    k = nc.dram_tensor("k", [B, H, lc, D], F32, kind="ExternalInput").ap()
    v = nc.dram_tensor("v", [B, H, lc, D], F32, kind="ExternalInput").ap()
    out = nc.dram_tensor("out", [2, B, H, lc, D], F32, kind="ExternalOutput").ap()

    n_chunks = lc // LCHUNK
    n_hg = H // HG

    with tile.TileContext(nc) as tc, ExitStack() as ctx:
        xpool = ctx.enter_context(tc.tile_pool(name="x", bufs=12))
        tpool = ctx.enter_context(tc.tile_pool(name="t", bufs=4))
        opool = ctx.enter_context(tc.tile_pool(name="o", bufs=5))
        ppool = ctx.enter_context(tc.tile_pool(name="p", bufs=2))
        cpool = ctx.enter_context(tc.tile_pool(name="c", bufs=2))

        n_groups = 2 * n_chunks
        group_no = 0
        tile_no = 0
        for ci, src in enumerate((k, v)):
            for lchunk in range(n_chunks):
                l0 = lchunk * LCHUNK
                # ---- load + per-tile partial min/max --------------------
                pmin = ppool.tile([128, B * n_hg], F32, tag="pmin")
                pmax = ppool.tile([128, B * n_hg], F32, tag="pmax")
                tiles = []
                j = 0
                for b in range(B):
                    for hg in range(n_hg):
                        x2 = xpool.tile([128, HG * D], F32, tag="x")
                        x3 = x2[:].rearrange("l (h d) -> l h d", h=HG)
                        src_ap = src[b, hg * HG:(hg + 1) * HG,
                                     l0:l0 + LCHUNK, :].rearrange("h l d -> l h d")
                        nc.sync.dma_start(out=x3, in_=src_ap)
                        nc.vector.tensor_reduce(pmin[:, j:j + 1], x2[:],
                                                axis=AXIS.X, op=ALU.min)
                        nc.vector.tensor_reduce(pmax[:, j:j + 1], x2[:],
                                                axis=AXIS.X, op=ALU.max)
                        tiles.append((x2, b, hg))
                        j += 1

                # ---- per-l constants (all [128,1]) ----------------------
                mn1 = cpool.tile([128, 1], F32, tag="mn1")
                mx1 = cpool.tile([128, 1], F32, tag="mx1")
                nc.vector.tensor_reduce(mn1[:], pmin[:], axis=AXIS.X, op=ALU.min)
                nc.vector.tensor_reduce(mx1[:], pmax[:], axis=AXIS.X, op=ALU.max)
                dd = cpool.tile([128, 1], F32, tag="dd")
                nc.vector.tensor_tensor(dd[:], mx1[:], mn1[:], op=ALU.subtract)
                s1 = cpool.tile([128, 1], F32, tag="s1")
                # s1 = max(d,1e-6) * (1/15) -- HW tensor_scalar has no divide;
                # differs from the reference's d/15 by <=1 ulp (rare boundary flips)
                nc.vector.tensor_scalar(s1[:], dd[:], 1e-6, C15,
                                        op0=ALU.max, op1=ALU.mult)
                inv1 = cpool.tile([128, 1], F32, tag="inv1")
                nc.vector.reciprocal(inv1[:], s1[:])
                a8 = cpool.tile([128, 1], F32, tag="a8")
                nc.vector.tensor_scalar(a8[:], s1[:], 8.0, None, op0=ALU.mult)
                z1 = cpool.tile([128, 1], F32, tag="z1")
                nc.vector.tensor_tensor(z1[:], mn1[:], a8[:], op=ALU.add)
                mn2 = cpool.tile([128, 1], F32, tag="mn2")
                nc.vector.tensor_tensor(mn2[:], z1[:], a8[:], op=ALU.subtract)
                b7 = cpool.tile([128, 1], F32, tag="b7")
                nc.vector.tensor_scalar(b7[:], s1[:], 7.0, None, op0=ALU.mult)
                mx2 = cpool.tile([128, 1], F32, tag="mx2")
                nc.vector.tensor_tensor(mx2[:], z1[:], b7[:], op=ALU.add)
                d2 = cpool.tile([128, 1], F32, tag="d2")
                nc.vector.tensor_tensor(d2[:], mx2[:], mn2[:], op=ALU.subtract)
                s2 = cpool.tile([128, 1], F32, tag="s2")
                nc.vector.tensor_scalar(s2[:], d2[:], 1e-6, C15,
                                        op0=ALU.max, op1=ALU.mult)
                nb1 = cpool.tile([128, 1], F32, tag="nb1")
                # nb1 = -(mn1 * inv1): bias for the fused ACT affine
                nc.vector.tensor_scalar(nb1[:], mn1[:], inv1[:, 0:1], -1.0,
                                        op0=ALU.mult, op1=ALU.mult)

                # ---- elementwise + store -------------------------------
                # stage1 (fused affine) + stage3 (fused affine) on ACT,
                # stage2 (magic round-to-nearest-even) on DVE, in place.
                # GPSIMD's stock ts/tt ucode measured ~15x slower than DVE,
                # and its SBUF-port sharing stalls DVE — keep Pool idle.
                # The last two groups run their affines on DVE instead:
                # at the tail DVE is idle while ACT is the critical path.
                tail = group_no >= n_groups - 2
                for x2, b, hg in tiles:
                    t = tpool.tile([128, HG * D], F32, tag="t")
                    if tail:
                        nc.vector.tensor_scalar(t[:], x2[:], mn1[:, 0:1],
                                                inv1[:, 0:1],
                                                op0=ALU.subtract, op1=ALU.mult)
                    else:
                        nc.scalar.activation(t[:], x2[:], ACTF.Identity,
                                             bias=nb1[:, 0:1], scale=inv1[:, 0:1])
                    # stage2 (round, magic-constant): one DVE ts, in place.
                    # (Tried as two chained ACT Identity adds for early
                    # tiles — measured slower; ACT's per-op cost dominates.)
                    nc.vector.tensor_scalar(t[:], t[:], MAGIC, MAGIC,
                                            op0=ALU.add, op1=ALU.subtract)
                    o = opool.tile([128, HG * D], F32, tag="o")
                    if tail:
                        nc.vector.tensor_scalar(o[:], t[:], s2[:, 0:1],
                                                mn2[:, 0:1],
                                                op0=ALU.mult, op1=ALU.add)
                    else:
                        nc.scalar.activation(o[:], t[:], ACTF.Identity,
                                             bias=mn2[:, 0:1], scale=s2[:, 0:1])
                    tile_no += 1
                    dst_ap = out[ci, b, hg * HG:(hg + 1) * HG,
                                 l0:l0 + LCHUNK, :].rearrange("h l d -> l h d")
                    nc.scalar.dma_start(
                        out=dst_ap,
                        in_=o[:].rearrange("l (h d) -> l h d", h=HG))
                group_no += 1

    nc.compile()
    return nc


def _get_nc(lc=LC):
    if lc not in _BUILD_CACHE:
        _BUILD_CACHE[lc] = _build(lc)
    return _BUILD_CACHE[lc]


def _host_fix_rows(out, cache_idx, val, input_pos):
    """Exact (fp32, reference-op-order) outputs for the scattered rows."""
    f32 = np.float32
    val = np.asarray(val, dtype=np.float32)
    pos = [int(p) for p in np.asarray(input_pos)]
    # last write wins for duplicate positions
    posmap = {}
    for i, p in enumerate(pos):
        posmap[p] = i
    for p, i in posmap.items():
        row = val[:, :, i, :]                       # [B,H,D]
        mn = row.min()
        mx = row.max()
        s2 = f32(max(mx - mn, f32(1e-6)) / f32(15))
        z2 = f32(mn + f32(s2 * f32(8)))
        t = ((row - mn) / s2).astype(np.float32)
        q = np.clip(np.round(t), 0, 15).astype(np.float32)
        out[cache_idx, :, :, p, :] = ((q - f32(8)) * s2).astype(np.float32) + z2


def kernel(k_cache_f, v_cache_f, k_val, v_val, input_pos):
    k_cache_f = np.asarray(k_cache_f, dtype=np.float32)
    v_cache_f = np.asarray(v_cache_f, dtype=np.float32)
    nc = _get_nc()
    in_maps = []
    for c in range(N_CORES):
        sl = slice(c * LC, (c + 1) * LC)
        in_maps.append({
            "k": np.ascontiguousarray(k_cache_f[:, :, sl, :]),
            "v": np.ascontiguousarray(v_cache_f[:, :, sl, :]),
        })
    res = run_bass_kernel_spmd(nc, in_maps, list(range(N_CORES)))
    out = np.concatenate([res.results[c]["out"] for c in range(N_CORES)], axis=3)
    _host_fix_rows(out, 0, k_val, input_pos)
    _host_fix_rows(out, 1, v_val, input_pos)
    return out

